# revision 12
# baseline (speedup 1.0000x reference)
"""Llama attention layer (B=2, S=2048, D=2048, H=16, fp32) on 8 Trainium2 cores.

Sharding: core c -> (batch b = c//4, head-group hg = c%4, 4 heads of 128 dims).
Column-parallel wq/wk/wv ([D, 512] slices), row-parallel wo ([512, D] slice);
host sums the 4 partial outputs per batch.

Per-core pipeline:
  Phase A: PE-transpose x -> xT slices; QKV projections (fp32r matmuls);
           RoPE on qT/kT (DVE, transposed layout); stage qT/kT/v to DRAM.
  Phase B: per head, causal scores S^T[j,i] = k_j . q_i via one 128-contraction
           matmul per block; unsafe softmax (no max subtract -- scores ~N(0,1));
           exp on ACT with fused 1/sqrt(128) scale; denominator via ones-
           stationary matmul accumulated alongside P@V; scale by reciprocal.
  Phase C: partial out-projection O = outT^T @ wo_slice, PSUM -> DRAM.
"""

import math
import sys

import numpy as np

sys.path.insert(0, "/opt/trn_rl_repo")

import concourse.bass as bass
import concourse.mybir as mybir
from concourse import bacc, bass_utils
from concourse.masks import make_identity
from concourse.tile import TileContext

B, S, D, H = 2, 2048, 2048, 16
HD = 128                 # head dim
NH = 4                   # heads per core
HG = NH * HD             # 512: q/k/v columns per core
NCORES = 8
KT = D // 128            # 16 contraction tiles
SB = 4                   # phase-A s-blocks
SBS = S // SB            # 512
QG = 4                   # phase-B q-groups
QGS = S // QG            # 512
F32 = mybir.dt.float32
F32R = mybir.dt.float32r
USE_F32R = True
CD = F32R if USE_F32R else F32
SCALE = HD ** -0.5
THETA = 10000.0

_cache = {}


def _rope_tables():
    inv_freq = 1.0 / (THETA ** (np.arange(0, HD, 2, dtype=np.float32) / HD))
    t = np.arange(S, dtype=np.float32)
    freqs = np.einsum("s,d->sd", t, inv_freq)        # [S, HD/2]
    emb = np.concatenate([freqs, freqs], axis=-1)    # [S, HD]
    return np.cos(emb).T.copy(), np.sin(emb).T.copy()  # [HD, S]


def _build_nc():
    nc = bacc.Bacc(None, target_bir_lowering=False, debug=False)
    x = nc.dram_tensor("x", [S, D], F32, kind="ExternalInput")
    wq = nc.dram_tensor("wq", [D, HG], CD, kind="ExternalInput")
    wk = nc.dram_tensor("wk", [D, HG], CD, kind="ExternalInput")
    wv = nc.dram_tensor("wv", [D, HG], CD, kind="ExternalInput")
    wo = nc.dram_tensor("wo", [HG, D], CD, kind="ExternalInput")
    cosT = nc.dram_tensor("cosT", [HD, S], F32, kind="ExternalInput")
    sinT = nc.dram_tensor("sinT", [HD, S], F32, kind="ExternalInput")
    maskT = nc.dram_tensor("maskT", [128, 128], F32, kind="ExternalInput")
    out = nc.dram_tensor("out", [S, D], F32, kind="ExternalOutput")

    with TileContext(nc) as tc:
        with (
            tc.tile_pool(name="const", bufs=1) as cpool,
            tc.tile_pool(name="dram", bufs=1, space="DRAM") as dpool,
        ):
            ident = cpool.tile([128, 128], F32)
            make_identity(nc, ident)
            mT = cpool.tile([128, 128], F32)
            nc.sync.dma_start(mT, maskT[:, :])
            ones_f = cpool.tile([128, 128], F32)
            nc.gpsimd.memset(ones_f, 1.0)
            ones = cpool.tile([128, 128], CD)
            nc.vector.tensor_copy(ones, ones_f)
            cosb = cpool.tile([HD, S], F32)
            sinb = cpool.tile([HD, S], F32)
            nc.sync.dma_start(cosb, cosT[:, :])
            nc.sync.dma_start(sinb, sinT[:, :])

            qTd = dpool.tile([HG, S], CD)   # [512, 2048] DRAM scratch
            kTd = dpool.tile([HG, S], CD)
            vd = dpool.tile([S, HG], CD)

            # ---------------- Phase A: projections + RoPE ----------------
            with (
                tc.tile_pool(name="wpool", bufs=1) as wpool,
                tc.tile_pool(name="xin", bufs=3) as xinp,
                tc.tile_pool(name="xT", bufs=1) as xtp,
                tc.tile_pool(name="stage", bufs=4) as stp,
                tc.tile_pool(name="ptA", bufs=2, space="PSUM") as pta,
                tc.tile_pool(name="pacc", bufs=5, space="PSUM") as pacc,
            ):
                wqt = wpool.tile([128, KT, HG], CD, tag="wq")
                wkt = wpool.tile([128, KT, HG], CD, tag="wk")
                wvt = wpool.tile([128, KT, HG], CD, tag="wv")
                nc.sync.dma_start(wqt, wq.rearrange("(n p) d -> p n d", p=128))
                nc.sync.dma_start(wkt, wk.rearrange("(n p) d -> p n d", p=128))
                nc.sync.dma_start(wvt, wv.rearrange("(n p) d -> p n d", p=128))

                for sb in range(SB):
                    xts = xtp.tile([128, KT, SBS], CD, tag="xT")
                    for t in range(4):          # 128-row s sub-tiles
                        for kc in range(4):     # 512-col k chunks
                            xin = xinp.tile([128, 512], F32, tag="xin")
                            nc.sync.dma_start(
                                xin,
                                x[sb * SBS + t * 128: sb * SBS + (t + 1) * 128,
                                  kc * 512:(kc + 1) * 512])
                            pt = pta.tile([128, 512], F32, tag="pt")
                            for j in range(4):
                                nc.tensor.transpose(
                                    pt[:, j * 128:(j + 1) * 128],
                                    xin[:, j * 128:(j + 1) * 128], ident)
                            nc.vector.tensor_copy(
                                xts[:, 4 * kc:4 * kc + 4, t * 128:(t + 1) * 128],
                                pt.rearrange("p (j s) -> p j s", j=4))

                    for wt, dst in ((wqt, qTd), (wkt, kTd)):
                        for hh in range(NH):
                            pq = pacc.tile([128, SBS], F32, tag="pacc")
                            for kk in range(KT):
                                nc.tensor.matmul(
                                    pq,
                                    lhsT=wt[:, kk, hh * HD:(hh + 1) * HD],
                                    rhs=xts[:, kk, :],
                                    start=(kk == 0), stop=(kk == KT - 1))
                            # RoPE in [d, s] layout
                            qs = stp.tile([128, SBS], CD, tag="qstage")
                            tmp = stp.tile([128, SBS], F32, tag="rtmp")
                            cs = cosb[:, sb * SBS:(sb + 1) * SBS]
                            sn = sinb[:, sb * SBS:(sb + 1) * SBS]
                            nc.vector.tensor_mul(tmp[0:64], pq[64:128], sn[0:64])
                            nc.vector.tensor_mul(tmp[64:128], pq[0:64], sn[64:128])
                            nc.vector.tensor_mul(qs, pq, cs)
                            nc.vector.tensor_sub(qs[0:64], qs[0:64], tmp[0:64])
                            nc.vector.tensor_add(qs[64:128], qs[64:128], tmp[64:128])
                            nc.sync.dma_start(
                                dst[hh * HD:(hh + 1) * HD, sb * SBS:(sb + 1) * SBS], qs)

                    for t in range(4):  # v in natural [s, d] layout
                        pv = pacc.tile([128, HG], F32, tag="pacc")
                        for kk in range(KT):
                            nc.tensor.matmul(
                                pv,
                                lhsT=xts[:, kk, t * 128:(t + 1) * 128],
                                rhs=wvt[:, kk, :],
                                start=(kk == 0), stop=(kk == KT - 1))
                        vs = stp.tile([128, HG], CD, tag="vstage")
                        nc.scalar.copy(vs, pv)
                        nc.sync.dma_start(
                            vd[sb * SBS + t * 128: sb * SBS + (t + 1) * 128, :], vs)

            # ---------------- Phase B: causal attention ----------------
            with (
                tc.tile_pool(name="outT", bufs=1) as otp,
                tc.tile_pool(name="wo", bufs=1) as wop,
            ):
                woT = wop.tile([128, NH, D], CD)
                nc.sync.dma_start(woT, wo.rearrange("(n p) d -> p n d", p=128))
                outT = otp.tile([128, NH, S], CD)

                with (
                    tc.tile_pool(name="kv", bufs=2) as kvp,
                    tc.tile_pool(name="expp", bufs=4) as expp,
                    tc.tile_pool(name="scl", bufs=3) as sclp,
                    tc.tile_pool(name="pst", bufs=4, space="PSUM") as pst,
                    tc.tile_pool(name="pout", bufs=2, space="PSUM") as pov,
                    tc.tile_pool(name="pden", bufs=2, space="PSUM") as pdn,
                ):
                    for h in range(NH):
                        kTh = kvp.tile([128, S], CD, tag="kT")
                        qTh = kvp.tile([128, S], CD, tag="qT")
                        vh = kvp.tile([128, KT, HD], CD, tag="v")
                        nc.sync.dma_start(kTh, kTd[h * HD:(h + 1) * HD, :])
                        nc.sync.dma_start(qTh, qTd[h * HD:(h + 1) * HD, :])
                        nc.sync.dma_start(
                            vh,
                            vd.rearrange("(n p) d -> p n d", p=128)[:, :, h * HD:(h + 1) * HD])
                        for g in range(QG):
                            po = pov.tile([128, QGS], F32, tag="po")
                            pd = pdn.tile([128, QGS], F32, tag="pd")
                            njt = 4 * g + 4
                            for jj in range(njt):
                                qlo = max(0, (jj - 4 * g) * 128)
                                ps = pst.tile([128, QGS], F32, tag="ps")
                                nc.tensor.matmul(
                                    ps[:, qlo:],
                                    lhsT=kTh[:, jj * 128:(jj + 1) * 128],
                                    rhs=qTh[:, g * QGS + qlo:(g + 1) * QGS],
                                    start=True, stop=True)
                                if jj >= 4 * g:  # diagonal 128x128 sub-block
                                    nc.vector.tensor_add(
                                        ps[:, qlo:qlo + 128], ps[:, qlo:qlo + 128], mT)
                                es = expp.tile([128, QGS], CD, tag="es")
                                nc.scalar.activation(
                                    es[:, qlo:], ps[:, qlo:],
                                    mybir.ActivationFunctionType.Exp, scale=SCALE)
                                nc.tensor.matmul(
                                    po[:, qlo:],
                                    lhsT=vh[:, jj, :],
                                    rhs=es[:, qlo:],
                                    start=(jj == 0), stop=(jj == njt - 1))
                                nc.tensor.matmul(
                                    pd[:, qlo:],
                                    lhsT=ones,
                                    rhs=es[:, qlo:],
                                    start=(jj == 0), stop=(jj == njt - 1))
                            rc = sclp.tile([128, QGS], F32, tag="rc")
                            nc.vector.reciprocal(rc, pd)
                            nc.vector.tensor_mul(
                                outT[:, h, g * QGS:(g + 1) * QGS], po, rc)

                # ---------------- Phase C: out projection ----------------
                with (
                    tc.tile_pool(name="pC", bufs=6, space="PSUM") as pcp,
                    tc.tile_pool(name="stC", bufs=4) as stc,
                ):
                    for st in range(16):
                        for nb in range(4):
                            pc = pcp.tile([128, 512], F32, tag="pc")
                            for h in range(NH):
                                nc.tensor.matmul(
                                    pc,
                                    lhsT=outT[:, h, st * 128:(st + 1) * 128],
                                    rhs=woT[:, h, nb * 512:(nb + 1) * 512],
                                    start=(h == 0), stop=(h == NH - 1))
                            oc = stc.tile([128, 512], F32, tag="oc")
                            nc.vector.tensor_copy(oc, pc)
                            nc.sync.dma_start(
                                out[st * 128:(st + 1) * 128, nb * 512:(nb + 1) * 512], oc)
    nc.compile()
    return nc


def _get_nc():
    if "nc" not in _cache:
        _cache["nc"] = _build_nc()
    return _cache["nc"]


def make_in_maps(x, wq, wk, wv, wo):
    cosT, sinT = _rope_tables()
    j = np.arange(128)[:, None]
    i = np.arange(128)[None, :]
    maskT = np.where(j <= i, 0.0, -1e9).astype(np.float32)
    in_maps = []
    for c in range(NCORES):
        b, hg = c // 4, c % 4
        cols = slice(hg * HG, (hg + 1) * HG)
        in_maps.append({
            "x": np.ascontiguousarray(x[b]),
            "wq": np.ascontiguousarray(wq[:, cols]),
            "wk": np.ascontiguousarray(wk[:, cols]),
            "wv": np.ascontiguousarray(wv[:, cols]),
            "wo": np.ascontiguousarray(wo[cols, :]),
            "cosT": cosT,
            "sinT": sinT,
            "maskT": maskT,
        })
    return in_maps


def run(x, wq, wk, wv, wo, **run_kwargs):
    nc = _get_nc()
    in_maps = make_in_maps(x, wq, wk, wv, wo)
    res = bass_utils.run_bass_kernel_spmd(
        nc, in_maps, core_ids=list(range(NCORES)), **run_kwargs)
    parts = np.stack([res.results[c]["out"] for c in range(NCORES)])
    out = np.empty((B, S, D), np.float32)
    for b in range(B):
        out[b] = parts[4 * b:4 * b + 4].sum(axis=0, dtype=np.float64).astype(np.float32)
    return out, res


def kernel(x, wq, wk, wv, wo, mask=None, **_ignored):
    out, _ = run(np.asarray(x), np.asarray(wq), np.asarray(wk),
                 np.asarray(wv), np.asarray(wo))
    return out
